# revision 1
# baseline (speedup 1.0000x reference)
"""BEV deformable-attention encoder layer on 8 Trainium2 NeuronCores.

Sharding: one offset-group/head per core (tensor-parallel over the (b*g)=8
leading dim). Host does the tiny irregular prep (offset conv network,
bilinear grid-sample, q/k/v grouped 1x1 projections ~3% of FLOPs); each core
runs the dominant compute: the CPB pairwise MLP (2->64->64->1 over
1600*100 pairs, ~1.3 GFLOP/core), attention logits, softmax, attn@V and its
partial slice of the final 1x1 output projection. Host sums the 8 partial
projections (the tensor-parallel unshard) and adds b_out.

CPB trick: layer-2 of the bias MLP is a matmul with lhsT = w2 placed in
column j of an otherwise-zero (64,100) matrix, accumulated straight into the
(100 j-part, q-free) attention-logit PSUM tile -> the bias lands pre-added to
q@k^T with no elementwise scatter at all.
"""

import math
import numpy as np

D_MODEL, HEADS, GROUPS, DIM_HEAD = 256, 8, 8, 64
INNER = HEADS * DIM_HEAD
OFF_DIMS = INNER // GROUPS
DF, OFF_SCALE, KS, PAD = 4, 4.0, 6, 1
NUM_LAYERS = 6
SCALE = DIM_HEAD ** -0.5
B, H, W = 1, 40, 40
HP = WP = 10
I, J = H * W, HP * WP       # 1600 queries, 100 keys
N_CORES = 8

_erf = np.frompyfunc(math.erf, 1, 1)


def _gelu_exact(x):
    return 0.5 * x * (1.0 + _erf(x / math.sqrt(2.0)).astype(np.float64)).astype(np.float32)


def _depthwise_conv(q_sp, w1, b1):
    # q_sp (64,40,40); w1 (64,1,6,6); stride 4 pad 1 -> (64,10,10)
    qp = np.zeros((OFF_DIMS, H + 2 * PAD, W + 2 * PAD), np.float32)
    qp[:, PAD:PAD + H, PAD:PAD + W] = q_sp
    out = np.zeros((OFF_DIMS, HP, WP), np.float32)
    for ky in range(KS):
        for kx in range(KS):
            out += qp[:, ky:ky + 4 * HP:DF, kx:kx + 4 * WP:DF] * w1[:, 0, ky, kx][:, None, None]
    return out + b1[:, None, None]


def _grid_sample(img, gxy):
    # img (C,40,40); gxy (J,2) normalized coords -> (C,J), zeros padding,
    # align_corners=False (faithful to reference)
    C = img.shape[0]
    gx = ((gxy[:, 0] + 1.0) * W - 1.0) * 0.5
    gy = ((gxy[:, 1] + 1.0) * H - 1.0) * 0.5
    x0 = np.floor(gx); y0 = np.floor(gy)
    wx1 = gx - x0; wy1 = gy - y0
    flat = img.reshape(C, H * W)
    out = np.zeros((C, gx.shape[0]), np.float32)
    for dx, dy, wgt in ((0, 0, (1 - wx1) * (1 - wy1)), (1, 0, wx1 * (1 - wy1)),
                        (0, 1, (1 - wx1) * wy1), (1, 1, wx1 * wy1)):
        xi = x0 + dx; yi = y0 + dy
        valid = (xi >= 0) & (xi <= W - 1) & (yi >= 0) & (yi <= H - 1)
        xc = np.clip(xi, 0, W - 1).astype(np.int32)
        yc = np.clip(yi, 0, H - 1).astype(np.int32)
        out += flat[:, yc * W + xc] * (wgt * valid).astype(np.float32)[None, :]
    return out


def _host_prep(bev_feat, wq, wk, wv, w_off1, b_off1, w_off2,
               cpb_w0, cpb_b0, cpb_w1, cpb_b1, cpb_w2, cpb_b2, w_out, b_out):
    """Everything tiny/irregular, in numpy. Returns per-core input dicts."""
    l = NUM_LAYERS - 1
    x = np.asarray(bev_feat, np.float32)[0].reshape(D_MODEL, I)      # (256,1600)

    # static query grid, normalized (channel0/x scaled by (H-1), ch1/y by (W-1))
    ys, xs = np.meshgrid(np.arange(H, dtype=np.float32),
                         np.arange(W, dtype=np.float32), indexing='ij')
    gq = np.stack([2.0 * xs / (H - 1) - 1.0, 2.0 * ys / (W - 1) - 1.0],
                  axis=-1).reshape(I, 2)                              # (1600,2)
    ysp, xsp = np.meshgrid(np.arange(HP, dtype=np.float32),
                           np.arange(WP, dtype=np.float32), indexing='ij')
    base_grid = np.stack([xsp, ysp])                                  # (2,10,10)

    ident = np.eye(128, dtype=np.float32)
    cores = []
    for g in range(GROUPS):
        xg = x[32 * g:32 * g + 32]                                    # (32,1600)
        q_g = np.asarray(wq[l][64 * g:64 * g + 64], np.float32) @ xg  # (64,1600)
        h = _depthwise_conv(q_g.reshape(OFF_DIMS, H, W),
                            np.asarray(w_off1[l], np.float32),
                            np.asarray(b_off1[l], np.float32))
        h = _gelu_exact(h).reshape(OFF_DIMS, J)
        off = np.tanh(np.asarray(w_off2[l], np.float32) @ h) * OFF_SCALE  # (2,J)
        vg = base_grid.reshape(2, J) + off
        gkv = np.stack([2.0 * vg[0] / (HP - 1) - 1.0,
                        2.0 * vg[1] / (WP - 1) - 1.0], axis=-1)       # (J,2)
        kv = _grid_sample(xg.reshape(32, H, W), gkv)                  # (32,J)
        k_g = np.asarray(wk[l][64 * g:64 * g + 64], np.float32) @ kv  # (64,J)
        v_g = np.asarray(wv[l][64 * g:64 * g + 64], np.float32) @ kv
        pos = gq[None, :, :] - gkv[:, None, :]                        # (J,I,2)
        xb = (np.sign(pos) * np.log1p(np.abs(pos))).astype(np.float32)
        xb2 = xb.transpose(2, 0, 1).reshape(2, J * I).copy()          # j-major
        w2 = np.asarray(cpb_w2[l], np.float32)[0]                     # (64,)
        w2s = np.zeros((OFF_DIMS, J, J), np.float32)
        w2s[:, np.arange(J), np.arange(J)] = w2[:, None]              # col j = w2
        cores.append({
            'qs': np.ascontiguousarray(q_g * SCALE),
            'k': np.ascontiguousarray(k_g),
            'vT': np.ascontiguousarray(v_g.T),                        # (J,64)
            'xb2': xb2,
            'w0T': np.ascontiguousarray(np.asarray(cpb_w0[l], np.float32).T),  # (2,64)
            'w1T': np.ascontiguousarray(np.asarray(cpb_w1[l], np.float32).T),  # (64,64)
            'w2s': np.ascontiguousarray(w2s.reshape(OFF_DIMS, J * J)),
            'b0': np.asarray(cpb_b0[l], np.float32).reshape(OFF_DIMS, 1).copy(),
            'b1': np.asarray(cpb_b1[l], np.float32).reshape(OFF_DIMS, 1).copy(),
            'woutT': np.ascontiguousarray(np.asarray(w_out[l], np.float32)[:, 64 * g:64 * g + 64].T),
            'ident': ident,
        })
    return cores, np.asarray(b_out[l], np.float32)


def _build_bass():
    import concourse.bass as bass
    import concourse.mybir as mybir
    from concourse.tile import TileContext

    f32 = mybir.dt.float32
    AF = mybir.ActivationFunctionType
    ALU = mybir.AluOpType
    AX = mybir.AxisListType

    nc = bass.Bass()
    d_qs = nc.dram_tensor('qs', [64, I], f32, kind='ExternalInput')
    d_k = nc.dram_tensor('k', [64, J], f32, kind='ExternalInput')
    d_vT = nc.dram_tensor('vT', [J, 64], f32, kind='ExternalInput')
    d_xb2 = nc.dram_tensor('xb2', [2, J * I], f32, kind='ExternalInput')
    d_w0T = nc.dram_tensor('w0T', [2, 64], f32, kind='ExternalInput')
    d_w1T = nc.dram_tensor('w1T', [64, 64], f32, kind='ExternalInput')
    d_w2s = nc.dram_tensor('w2s', [64, J * J], f32, kind='ExternalInput')
    d_b0 = nc.dram_tensor('b0', [64, 1], f32, kind='ExternalInput')
    d_b1 = nc.dram_tensor('b1', [64, 1], f32, kind='ExternalInput')
    d_woutT = nc.dram_tensor('woutT', [64, D_MODEL], f32, kind='ExternalInput')
    d_ident = nc.dram_tensor('ident', [128, 128], f32, kind='ExternalInput')
    d_P = nc.dram_tensor('P', [D_MODEL, I], f32, kind='ExternalOutput')

    WINDOWS = [(0, 500), (500, 500), (1000, 500), (1500, 100)]

    with TileContext(nc) as tc:
        with tc.tile_pool(name='const', bufs=1) as cpool, \
             tc.tile_pool(name='work', bufs=4) as wpool, \
             tc.tile_pool(name='big', bufs=2) as bpool, \
             tc.tile_pool(name='pm', bufs=2, space='PSUM') as pm, \
             tc.tile_pool(name='pa', bufs=2, space='PSUM') as pa:

            qs_t = cpool.tile([64, I], f32, tag='qs')
            nc.sync.dma_start(out=qs_t[:], in_=d_qs[:])
            k_t = cpool.tile([64, J], f32, tag='k')
            nc.sync.dma_start(out=k_t[:], in_=d_k[:])
            vT_t = cpool.tile([J, 64], f32, tag='vT')
            nc.sync.dma_start(out=vT_t[:], in_=d_vT[:])
            w0T_t = cpool.tile([2, 64], f32, tag='w0T')
            nc.sync.dma_start(out=w0T_t[:], in_=d_w0T[:])
            w1T_t = cpool.tile([64, 64], f32, tag='w1T')
            nc.sync.dma_start(out=w1T_t[:], in_=d_w1T[:])
            w2s_t = cpool.tile([64, J * J], f32, tag='w2s')
            nc.sync.dma_start(out=w2s_t[:], in_=d_w2s[:])
            b0_t = cpool.tile([64, 1], f32, tag='b0')
            nc.sync.dma_start(out=b0_t[:], in_=d_b0[:])
            b1_t = cpool.tile([64, 1], f32, tag='b1')
            nc.sync.dma_start(out=b1_t[:], in_=d_b1[:])
            woutT_t = cpool.tile([64, D_MODEL], f32, tag='woutT')
            nc.sync.dma_start(out=woutT_t[:], in_=d_woutT[:])
            id_t = cpool.tile([128, 128], f32, tag='ident')
            nc.sync.dma_start(out=id_t[:], in_=d_ident[:])
            outT_s = cpool.tile([64, I], f32, tag='outT')

            for (w0, m) in WINDOWS:
                simTp = pa.tile([J, 500], f32, tag='simT')
                # attention logits q@k^T, transposed: (j, q)
                nc.tensor.matmul(simTp[:, :m], k_t[:], qs_t[:, w0:w0 + m],
                                 start=True, stop=False)
                for j in range(J):
                    xbt = wpool.tile([2, 500], f32, tag='xbt')
                    nc.sync.dma_start(out=xbt[:, :m],
                                      in_=d_xb2[:, j * I + w0: j * I + w0 + m])
                    h1p = pm.tile([64, 500], f32, tag='h1p')
                    nc.tensor.matmul(h1p[:, :m], w0T_t[:], xbt[:, :m],
                                     start=True, stop=True)
                    h1s = wpool.tile([64, 500], f32, tag='h1s')
                    nc.scalar.activation(h1s[:, :m], h1p[:, :m], AF.Relu,
                                         bias=b0_t[:], scale=1.0)
                    h2p = pm.tile([64, 500], f32, tag='h2p')
                    nc.tensor.matmul(h2p[:, :m], w1T_t[:], h1s[:, :m],
                                     start=True, stop=True)
                    h2s = wpool.tile([64, 500], f32, tag='h2s')
                    nc.vector.tensor_scalar(h2s[:, :m], h2p[:, :m], b1_t[:], 0.0,
                                            op0=ALU.add, op1=ALU.max)
                    # CPB layer 2, accumulated into logits at row j
                    nc.tensor.matmul(simTp[:, :m], w2s_t[:, j * J:(j + 1) * J],
                                     h2s[:, :m], start=False, stop=(j == J - 1))

                simTs = bpool.tile([J, 500], f32, tag='simTs')
                nc.vector.tensor_copy(simTs[:, :m], simTp[:, :m])
                for s0 in range(0, m, 125):
                    sl = min(125, m - s0)
                    trp = pa.tile([128, J], f32, tag='trp')
                    nc.tensor.transpose(trp[:sl, :], simTs[:, s0:s0 + sl], id_t[:J, :J])
                    e_s = wpool.tile([128, J], f32, tag='es')
                    nc.scalar.activation(e_s[:sl, :], trp[:sl, :], AF.Exp)
                    ssum = wpool.tile([128, 1], f32, tag='ssum')
                    nc.vector.reduce_sum(ssum[:sl, :], e_s[:sl, :], axis=AX.X)
                    rec = wpool.tile([128, 1], f32, tag='rec')
                    nc.vector.reciprocal(rec[:sl, :], ssum[:sl, :])
                    nc.vector.tensor_scalar_mul(e_s[:sl, :], e_s[:sl, :], rec[:sl, :])
                    tr2 = pa.tile([J, 128], f32, tag='tr2')
                    nc.tensor.transpose(tr2[:, :sl], e_s[:sl, :J], id_t[:sl, :sl])
                    attTs = wpool.tile([J, 128], f32, tag='attTs')
                    nc.vector.tensor_copy(attTs[:, :sl], tr2[:, :sl])
                    outTp = pa.tile([64, 128], f32, tag='outTp')
                    nc.tensor.matmul(outTp[:, :sl], vT_t[:], attTs[:, :sl],
                                     start=True, stop=True)
                    nc.scalar.copy(outT_s[:, w0 + s0:w0 + s0 + sl], outTp[:, :sl])

            # partial output projection: P = woutT.T @ outT  (256,1600)
            for half in range(2):
                for c in range(4):
                    pp = pa.tile([128, 400], f32, tag='pp')
                    nc.tensor.matmul(pp[:], woutT_t[:, 128 * half:128 * half + 128],
                                     outT_s[:, 400 * c:400 * c + 400],
                                     start=True, stop=True)
                    ps = wpool.tile([128, 400], f32, tag='ps')
                    nc.vector.tensor_copy(ps[:], pp[:])
                    nc.sync.dma_start(
                        out=d_P[128 * half:128 * half + 128, 400 * c:400 * c + 400],
                        in_=ps[:])
    return nc


_NC_CACHE = {}


def _run_device(cores):
    from concourse.bass_utils import run_bass_kernel_spmd
    if 'nc' not in _NC_CACHE:
        _NC_CACHE['nc'] = _build_bass()
    nc = _NC_CACHE['nc']
    res = run_bass_kernel_spmd(nc, cores, core_ids=list(range(N_CORES)))
    return [r['P'] for r in res.results]


def _cpb_attn_numpy(cores):
    """Fallback: same per-core math in numpy."""
    outs = []
    for cin in cores:
        xb = cin['xb2'].reshape(2, J, I)
        h1 = np.maximum(np.einsum('co,cji->oji', cin['w0T'], xb) + cin['b0'][:, :, None], 0.0)
        h2 = np.maximum(np.einsum('co,cji->oji', cin['w1T'], h1) + cin['b1'][:, :, None], 0.0)
        w2 = cin['w2s'].reshape(64, J, J)[:, 0, 0][:, None, None] * 0
        w2v = np.array([cin['w2s'].reshape(64, J, J)[c, 0, 0] for c in range(64)], np.float32)
        bias = np.einsum('c,cji->ji', w2v, h2)                       # (J,I)
        sim = cin['k'].T @ cin['qs'] + bias                           # (J,I)
        sim = sim - sim.max(axis=0, keepdims=True)
        e = np.exp(sim)
        att = e / e.sum(axis=0, keepdims=True)                        # (J,I)
        outT = cin['vT'].T @ att                                      # (64,I)
        outs.append(cin['woutT'].T @ outT)                            # (256,I)
    return outs


def kernel(**inputs):
    cores, b_out = _host_prep(**inputs)
    try:
        parts = _run_device(cores)
    except Exception as e:  # last-resort correctness fallback
        import traceback; traceback.print_exc()
        parts = _cpb_attn_numpy(cores)
    acc = np.zeros((D_MODEL, I), np.float32)
    for p in parts:
        acc += p
    acc += b_out[:, None]
    return acc.reshape(1, D_MODEL, H, W).astype(np.float32)



# revision 2
# speedup vs baseline: 1.6299x; 1.6299x over previous
"""BEV deformable-attention encoder layer on 8 Trainium2 NeuronCores.

Sharding: one offset-group/head per core (tensor-parallel over the (b*g)=8
leading dim, per the sharding hint). Host does only the tiny irregular prep
(q/k/v grouped 1x1 projections, the 6x6 stride-4 offset conv + GELU + tanh,
bilinear grid-sample -- together ~3% of FLOPs); each core runs the dominant
compute: the CPB pairwise MLP (2->64->64->1 over 1600*100 pairs,
~1.4 GFLOP/core), attention logits, softmax, attn@V and its partial slice of
the final 1x1 output projection. Host sums the 8 partial projections and adds
b_out.

Device pipeline (bf16 matmuls, fp32 PSUM accumulation):
- pairs are processed "2 j's at a time" so every MLP matmul uses the full
  128-wide partition dim:
    L1: lhsT (4,128)   = blockdiag(w0, w0), rhs = packed features (4, N)
    L2: lhsT (128,128) = blockdiag(w1^T, w1^T)
    L3: lhsT (128,100) = w2 placed in column j1 (rows 0-63) / j2 (rows 64-127),
        accumulated straight into the (100 j, N i) attention-logit PSUM tile
        on top of q@k^T -> the bias lands pre-added with no elementwise pass.
- softmax over j (partition dim) via PE transpose chunks; exp+rowsum fused in
  one ACT instruction (accum_out); cpb_b2 is dropped (constant shift over j,
  softmax-invariant).
"""

import math
import numpy as np
import ml_dtypes

BF16 = ml_dtypes.bfloat16

D_MODEL, HEADS, GROUPS, DIM_HEAD = 256, 8, 8, 64
INNER = HEADS * DIM_HEAD
OFF_DIMS = INNER // GROUPS            # 64
DF, OFF_SCALE, KS, PAD = 4, 4.0, 6, 1
NUM_LAYERS = 6
SCALE = DIM_HEAD ** -0.5
B, H, W = 1, 40, 40
HP = WP = 10
I, J = H * W, HP * WP                 # 1600 queries, 100 keys
JP = J // 2                           # 50 j-pairs
NW, WN = 4, 400                       # 4 windows of 400 queries
N_CORES = 8


def _gelu_exact(x):
    from scipy.special import erf
    return 0.5 * x * (1.0 + erf(x / math.sqrt(2.0)))


def _depthwise_conv(q_sp, w1, b1):
    # q_sp (64,40,40); w1 (64,1,6,6); stride 4 pad 1 -> (64,10,10)
    qp = np.zeros((OFF_DIMS, H + 2 * PAD, W + 2 * PAD), np.float32)
    qp[:, PAD:PAD + H, PAD:PAD + W] = q_sp
    out = np.zeros((OFF_DIMS, HP, WP), np.float32)
    for ky in range(KS):
        for kx in range(KS):
            out += qp[:, ky:ky + 4 * HP:DF, kx:kx + 4 * WP:DF] * w1[:, 0, ky, kx][:, None, None]
    return out + b1[:, None, None]


def _grid_sample(img, gxy):
    # img (C,40,40); gxy (J,2) normalized coords -> (C,J); zeros padding,
    # align_corners=False (faithful to reference)
    C = img.shape[0]
    gx = ((gxy[:, 0] + 1.0) * W - 1.0) * 0.5
    gy = ((gxy[:, 1] + 1.0) * H - 1.0) * 0.5
    x0 = np.floor(gx); y0 = np.floor(gy)
    wx1 = gx - x0; wy1 = gy - y0
    flat = img.reshape(C, H * W)
    out = np.zeros((C, gx.shape[0]), np.float32)
    for dx, dy, wgt in ((0, 0, (1 - wx1) * (1 - wy1)), (1, 0, wx1 * (1 - wy1)),
                        (0, 1, (1 - wx1) * wy1), (1, 1, wx1 * wy1)):
        xi = x0 + dx; yi = y0 + dy
        valid = (xi >= 0) & (xi <= W - 1) & (yi >= 0) & (yi <= H - 1)
        xc = np.clip(xi, 0, W - 1).astype(np.int32)
        yc = np.clip(yi, 0, H - 1).astype(np.int32)
        out += flat[:, yc * W + xc] * (wgt * valid).astype(np.float32)[None, :]
    return out


def _host_prep(bev_feat, wq, wk, wv, w_off1, b_off1, w_off2,
               cpb_w0, cpb_b0, cpb_w1, cpb_b1, cpb_w2, cpb_b2, w_out, b_out):
    """Everything tiny/irregular, in numpy. Returns per-core input dicts."""
    l = NUM_LAYERS - 1
    x = np.asarray(bev_feat, np.float32)[0].reshape(D_MODEL, I)      # (256,1600)

    # static query grid, normalized (channel0/x scaled by (H-1), ch1/y by (W-1))
    ys, xs = np.meshgrid(np.arange(H, dtype=np.float32),
                         np.arange(W, dtype=np.float32), indexing='ij')
    gq = np.stack([2.0 * xs / (H - 1) - 1.0, 2.0 * ys / (W - 1) - 1.0],
                  axis=-1).reshape(I, 2)                              # (1600,2)
    ysp, xsp = np.meshgrid(np.arange(HP, dtype=np.float32),
                           np.arange(WP, dtype=np.float32), indexing='ij')
    base_grid = np.stack([xsp, ysp])                                  # (2,10,10)

    w_off1_l = np.asarray(w_off1[l], np.float32)
    b_off1_l = np.asarray(b_off1[l], np.float32)
    w_off2_l = np.asarray(w_off2[l], np.float32)
    w0 = np.asarray(cpb_w0[l], np.float32)                            # (64,2)
    b0 = np.asarray(cpb_b0[l], np.float32)                            # (64,)
    w1 = np.asarray(cpb_w1[l], np.float32)                            # (64,64)
    b1 = np.asarray(cpb_b1[l], np.float32)                            # (64,)
    w2 = np.asarray(cpb_w2[l], np.float32)[0]                         # (64,)
    wq_l = np.asarray(wq[l], np.float32)
    wk_l = np.asarray(wk[l], np.float32)
    wv_l = np.asarray(wv[l], np.float32)
    w_out_l = np.asarray(w_out[l], np.float32)

    # constant device-side weight blocks (identical across cores)
    w0b = np.zeros((4, 128), np.float32)
    w0b[0, :64] = w0[:, 0]; w0b[1, :64] = w0[:, 1]
    w0b[2, 64:] = w0[:, 0]; w0b[3, 64:] = w0[:, 1]
    w1b = np.zeros((128, 128), np.float32)
    w1b[:64, :64] = w1.T; w1b[64:, 64:] = w1.T
    b0b = np.concatenate([b0, b0]).reshape(128, 1)
    b1b = np.concatenate([b1, b1]).reshape(128, 1)
    w2b = np.zeros((128, JP, J), np.float32)
    w2b[:64, np.arange(JP), 2 * np.arange(JP)] = w2[:, None]
    w2b[64:, np.arange(JP), 2 * np.arange(JP) + 1] = w2[:, None]
    w2b = w2b.reshape(128, JP * J)
    ident = np.eye(128, dtype=np.float32)

    const = {
        'w0b': w0b.astype(BF16), 'w1b': w1b.astype(BF16),
        'b0b': b0b, 'b1b': b1b,
        'w2b': w2b.astype(BF16), 'ident': ident.astype(BF16),
    }

    cores = []
    for g in range(GROUPS):
        xg = x[32 * g:32 * g + 32]                                    # (32,1600)
        q_g = wq_l[64 * g:64 * g + 64] @ xg                           # (64,1600)
        h = _depthwise_conv(q_g.reshape(OFF_DIMS, H, W), w_off1_l, b_off1_l)
        h = _gelu_exact(h).reshape(OFF_DIMS, J)
        off = np.tanh(w_off2_l @ h) * OFF_SCALE                       # (2,J)
        vg = base_grid.reshape(2, J) + off
        gkv = np.stack([2.0 * vg[0] / (HP - 1) - 1.0,
                        2.0 * vg[1] / (WP - 1) - 1.0], axis=-1)       # (J,2)
        kv = _grid_sample(xg.reshape(32, H, W), gkv)                  # (32,J)
        k_g = wk_l[64 * g:64 * g + 64] @ kv                           # (64,J)
        v_g = wv_l[64 * g:64 * g + 64] @ kv
        # CPB pairwise features, signed-log: F[c, j, i] = s(gq[i,c] - gkv[j,c])
        pos = gq.T[:, None, :] - gkv.T[:, :, None]                    # (2,J,I)
        F = np.sign(pos) * np.log1p(np.abs(pos))
        xb4 = np.stack([F[0, 0::2], F[1, 0::2], F[0, 1::2], F[1, 1::2]])
        cores.append({
            'xb4': np.ascontiguousarray(xb4.reshape(4, JP * I)).astype(BF16),
            'qs': (q_g * SCALE).astype(BF16),
            'k': np.ascontiguousarray(k_g).astype(BF16),
            'vT': np.ascontiguousarray(v_g.T).astype(BF16),           # (J,64)
            'woT': np.ascontiguousarray(w_out_l[:, 64 * g:64 * g + 64].T).astype(BF16),
            **const,
        })
    return cores, np.asarray(b_out[l], np.float32)


def _build_bass():
    import concourse.bass as bass
    import concourse.mybir as mybir
    from concourse.tile import TileContext

    f32 = mybir.dt.float32
    bf16 = mybir.dt.bfloat16
    AF = mybir.ActivationFunctionType
    ALU = mybir.AluOpType
    AX = mybir.AxisListType

    nc = bass.Bass()
    d_xb4 = nc.dram_tensor('xb4', [4, JP * I], bf16, kind='ExternalInput')
    d_qs = nc.dram_tensor('qs', [64, I], bf16, kind='ExternalInput')
    d_k = nc.dram_tensor('k', [64, J], bf16, kind='ExternalInput')
    d_vT = nc.dram_tensor('vT', [J, 64], bf16, kind='ExternalInput')
    d_woT = nc.dram_tensor('woT', [64, D_MODEL], bf16, kind='ExternalInput')
    d_w0b = nc.dram_tensor('w0b', [4, 128], bf16, kind='ExternalInput')
    d_w1b = nc.dram_tensor('w1b', [128, 128], bf16, kind='ExternalInput')
    d_b0b = nc.dram_tensor('b0b', [128, 1], f32, kind='ExternalInput')
    d_b1b = nc.dram_tensor('b1b', [128, 1], f32, kind='ExternalInput')
    d_w2b = nc.dram_tensor('w2b', [128, JP * J], bf16, kind='ExternalInput')
    d_ident = nc.dram_tensor('ident', [128, 128], bf16, kind='ExternalInput')
    d_P = nc.dram_tensor('P', [D_MODEL, I], f32, kind='ExternalOutput')

    with TileContext(nc) as tc:
        with tc.tile_pool(name='const', bufs=1) as cpool, \
             tc.tile_pool(name='work', bufs=3) as wpool, \
             tc.tile_pool(name='soft', bufs=2) as spool, \
             tc.tile_pool(name='pmm', bufs=2, space='PSUM') as pmm, \
             tc.tile_pool(name='pacc', bufs=1, space='PSUM') as pacc, \
             tc.tile_pool(name='paux', bufs=1, space='PSUM') as paux:

            def cload(name, dram, shape, dtype):
                t = cpool.tile(shape, dtype, tag=name)
                nc.sync.dma_start(out=t[:], in_=dram[:])
                return t

            xb4_t = cload('xb4', d_xb4, [4, JP * I], bf16)
            qs_t = cload('qs', d_qs, [64, I], bf16)
            k_t = cload('k', d_k, [64, J], bf16)
            vT_t = cload('vT', d_vT, [J, 64], bf16)
            woT_t = cload('woT', d_woT, [64, D_MODEL], bf16)
            w0b_t = cload('w0b', d_w0b, [4, 128], bf16)
            w1b_t = cload('w1b', d_w1b, [128, 128], bf16)
            b0b_t = cload('b0b', d_b0b, [128, 1], f32)
            b1b_t = cload('b1b', d_b1b, [128, 1], f32)
            w2b_t = cload('w2b', d_w2b, [128, JP * J], bf16)
            id_t = cload('ident', d_ident, [128, 128], bf16)

            outs = cpool.tile([64, I], bf16, tag='outs')   # attn out (d, i)

            for win in range(NW):
                w0c = win * WN
                simT = pacc.tile([J, WN], f32, tag='simT')
                # attention logits q@k^T, transposed: (j, i)
                nc.tensor.matmul(simT[:], k_t[:], qs_t[:, w0c:w0c + WN],
                                 start=True, stop=False)
                for jp in range(JP):
                    c0 = jp * I + w0c
                    h0p = pmm.tile([128, WN], f32, tag='h0p')
                    nc.tensor.matmul(h0p[:], w0b_t[:], xb4_t[:, c0:c0 + WN],
                                     start=True, stop=True)
                    h0s = wpool.tile([128, WN], bf16, tag='h0s')
                    nc.scalar.activation(h0s[:], h0p[:], AF.Relu, bias=b0b_t[:])
                    h1p = pmm.tile([128, WN], f32, tag='h1p')
                    nc.tensor.matmul(h1p[:], w1b_t[:], h0s[:],
                                     start=True, stop=True)
                    h1s = wpool.tile([128, WN], bf16, tag='h1s')
                    nc.vector.tensor_scalar(h1s[:], h1p[:], b1b_t[:], 0.0,
                                            op0=ALU.add, op1=ALU.max)
                    # CPB layer-3 accumulated into the logits at rows (2jp, 2jp+1)
                    nc.tensor.matmul(simT[:], w2b_t[:, jp * J:(jp + 1) * J],
                                     h1s[:], start=False, stop=(jp == JP - 1))

                # softmax over j (partition dim) via transpose chunks of 100
                simTs = spool.tile([J, WN], bf16, tag='simTs')
                nc.vector.tensor_copy(simTs[:], simT[:])
                for c in range(WN // J):
                    i0 = c * J
                    trp = paux.tile([J, J], f32, tag='trp')
                    nc.tensor.transpose(trp[:], simTs[:, i0:i0 + J], id_t[:J, :J])
                    es = wpool.tile([J, J], f32, tag='es')
                    ssum = wpool.tile([J, 1], f32, tag='ssum')
                    nc.scalar.activation(es[:], trp[:], AF.Exp, accum_out=ssum[:])
                    rec = wpool.tile([J, 1], f32, tag='rec')
                    nc.vector.reciprocal(rec[:], ssum[:])
                    esb = wpool.tile([J, J], bf16, tag='esb')
                    nc.vector.tensor_scalar_mul(esb[:], es[:], rec[:])
                    tr2 = paux.tile([J, J], f32, tag='tr2')
                    nc.tensor.transpose(tr2[:], esb[:], id_t[:J, :J])
                    attT = wpool.tile([J, J], bf16, tag='attT')
                    nc.vector.tensor_copy(attT[:], tr2[:])
                    op = paux.tile([64, J], f32, tag='mmout')
                    nc.tensor.matmul(op[:], vT_t[:], attT[:],
                                     start=True, stop=True)
                    nc.scalar.copy(outs[:, w0c + i0:w0c + i0 + J], op[:])

            # partial output projection: P = woT.T @ outs  (256,1600)
            for half in range(2):
                for cw in range(NW):
                    pp = paux.tile([128, WN], f32, tag='mmout')
                    nc.tensor.matmul(pp[:], woT_t[:, 128 * half:128 * half + 128],
                                     outs[:, WN * cw:WN * cw + WN],
                                     start=True, stop=True)
                    ps = wpool.tile([128, WN], f32, tag='ps')
                    nc.vector.tensor_copy(ps[:], pp[:])
                    nc.sync.dma_start(
                        out=d_P[128 * half:128 * half + 128, WN * cw:WN * cw + WN],
                        in_=ps[:])
    return nc


_NC_CACHE = {}


def _get_nc():
    if 'nc' not in _NC_CACHE:
        _NC_CACHE['nc'] = _build_bass()
    return _NC_CACHE['nc']


def _run_device(cores, trace=False, tmpdir=None):
    from concourse.bass_utils import run_bass_kernel_spmd
    res = run_bass_kernel_spmd(_get_nc(), cores, core_ids=list(range(N_CORES)),
                               trace=trace, tmpdir=tmpdir)
    return res


def _cpb_attn_numpy(cores):
    """Fallback: same per-core math in numpy (slow but exact)."""
    outs = []
    for cin in cores:
        xb4 = np.asarray(cin['xb4'], np.float32).reshape(4, JP, I)
        F = np.empty((2, J, I), np.float32)
        F[0, 0::2] = xb4[0]; F[1, 0::2] = xb4[1]
        F[0, 1::2] = xb4[2]; F[1, 1::2] = xb4[3]
        w0b = np.asarray(cin['w0b'], np.float32)
        w1 = np.asarray(cin['w1b'], np.float32)[:64, :64].T
        w2b = np.asarray(cin['w2b'], np.float32).reshape(128, JP, J)
        w2 = w2b[:64, 0, 0]
        b0 = cin['b0b'][:64, 0]; b1 = cin['b1b'][:64, 0]
        w0 = np.stack([w0b[0, :64], w0b[1, :64]], axis=1)             # (64,2)
        xb = F.reshape(2, J * I)
        h0 = np.maximum(w0 @ xb + b0[:, None], 0.0)
        h1 = np.maximum(w1 @ h0 + b1[:, None], 0.0)
        bias = (w2 @ h1).reshape(J, I)
        qs = np.asarray(cin['qs'], np.float32)
        k = np.asarray(cin['k'], np.float32)
        vT = np.asarray(cin['vT'], np.float32)
        woT = np.asarray(cin['woT'], np.float32)
        sim = k.T @ qs + bias                                         # (J,I)
        sim = sim - sim.max(axis=0, keepdims=True)
        e = np.exp(sim)
        att = e / e.sum(axis=0, keepdims=True)
        outT = vT.T @ att                                             # (64,I)
        outs.append(woT.T @ outT)                                     # (256,I)
    return outs


def kernel(**inputs):
    cores, b_out = _host_prep(**inputs)
    try:
        parts = [r['P'] for r in _run_device(cores).results]
    except Exception:  # last-resort correctness fallback
        import traceback; traceback.print_exc()
        parts = _cpb_attn_numpy(cores)
    acc = np.zeros((D_MODEL, I), np.float32)
    for p in parts:
        acc += np.asarray(p, np.float32)
    acc += b_out[:, None]
    return acc.reshape(1, D_MODEL, H, W).astype(np.float32)
